# revision 72
# baseline (speedup 1.0000x reference)
"""Trainium2 Bass kernel for nn_Decoder (Tacotron-style LSTM encoder/decoder).

Architecture (8 NeuronCores, data-parallel over batch N=64 -> 8/core):
  - Transposed ("World B") layout: hidden dim on SBUF partitions, (chunk,batch)
    on the free dim, so the h produced by the elementwise tail is directly the
    next step's matmul rhs (no per-step transposes).
  - Teacher forcing / layer chunking: input-side projections are batched into
    large matmuls per 64-step chunk; only h @ Whh.T stays per-step.
  - FULLY UNROLLED instruction stream (no hardware loops): loop-register
    updates (FusedRegOps, 96ns each on the PE sequencer) dominated the looped
    version; immediate addressing removes them entirely.
  - Gate order re-tiled host-side to i,f,o,g so one sigmoid instruction covers
    i/f/o and one tanh covers g.
  - Concurrent scans (encoder fwd+bwd; decoder wavefront layers grouped in
    pairs) share one PSUM gate bank so the elementwise tail is emitted once
    per group, not once per scan.
  - Weights / h / x-projections in bf16, cell state c and PSUM in fp32.
"""

import numpy as np
import ml_dtypes

H = 256
NMEL = 80
D_ENC = 512
NCORES = 8
NL = 8          # batch per core
C = 32          # chunk (time) size (smaller chunk = shorter wavefront fill)

# new gate order: i,f,o,g (old torch order i,f,g,o); 8 j-chunks of 128
GATE_PERM = [0, 1, 2, 3, 6, 7, 4, 5]

_prog_cache = {}


def _build_program(S, T):
    """Build the Bass program for full sequence length S (encoder) and T
    (mels length; decoder runs TD = T padded steps). Returns nc."""
    import concourse.bass as bass
    import concourse.mybir as mybir
    import concourse.tile as tile
    from concourse import bacc
    from concourse.bass import ds
    from concourse.masks import make_identity
    from contextlib import ExitStack

    BF = mybir.dt.bfloat16
    FP = mybir.dt.float32
    A = mybir.ActivationFunctionType

    TD = T
    assert S % C == 0 and TD % C == 0
    SC = S // C
    DC = TD // C

    nc = bacc.Bacc("TRN2", target_bir_lowering=False, debug=False,
                   num_devices=NCORES)

    # ---------------- DRAM I/O ----------------
    d_encrhs = nc.dram_tensor("encrhs", [4, 128, S, NL], BF, kind="ExternalInput").ap()
    d_mels = nc.dram_tensor("mels", [NL, NMEL, T], FP, kind="ExternalInput").ap()
    d_ictx = nc.dram_tensor("ictx", [NMEL], FP, kind="ExternalInput").ap()
    d_ewih = nc.dram_tensor("ewih", [128, 128, 128], BF, kind="ExternalInput").ap()
    d_ewhh = nc.dram_tensor("ewhh", [128, 64, 128], BF, kind="ExternalInput").ap()
    d_eb = nc.dram_tensor("eb", [1, 32, 128], BF, kind="ExternalInput").ap()
    d_dwih0 = nc.dram_tensor("dwih0", [97, 8, 128], BF, kind="ExternalInput").ap()
    d_dwih = nc.dram_tensor("dwih", [128, 48, 128], BF, kind="ExternalInput").ap()
    d_db = nc.dram_tensor("db", [1, 24, 128], BF, kind="ExternalInput").ap()
    d_dwhh = nc.dram_tensor("dwhh", [128, 64, 128], BF, kind="ExternalInput").ap()
    d_fcw = nc.dram_tensor("fcw", [128, 2, NMEL], BF, kind="ExternalInput").ap()
    d_fcb = nc.dram_tensor("fcb", [1, NMEL], BF, kind="ExternalInput").ap()
    d_out = nc.dram_tensor("out", [NL, NMEL, T], FP, kind="ExternalOutput").ap()
    out_r = d_out.rearrange("n c t -> c n t")  # [80, NL, T]

    CB = C * NL  # tokens per chunk = 512
    PS = 2 * NL  # per-scan gate-chunk block (kk-major x batch) = 16

    with tile.TileContext(nc) as tc:
        with ExitStack() as ctx:
            persist = ctx.enter_context(tc.tile_pool(name="persist", bufs=1))
            psum_x = ctx.enter_context(
                tc.tile_pool(name="psx", bufs=2, space="PSUM"))
            stash = ctx.enter_context(tc.tile_pool(name="stash", bufs=2))

            ident = persist.tile([128, 128], BF)
            make_identity(nc, ident)
            ones = persist.tile([1, CB], BF)
            nc.vector.memset(ones, 1.0)

            # decoder init states copied out of encoder scope
            hinit = [persist.tile([128, PS], BF, tag=f"hi{l}", name=f"hinit{l}")
                     for l in range(4)]
            cinit = [persist.tile([128, PS], FP, tag=f"ci{l}", name=f"cinit{l}")
                     for l in range(4)]

            # ----------------------------------------------------------------
            # A "group" merges the elementwise tail of several concurrent
            # scans.  Group PSUM layout: per scan slot a, a contiguous
            # 4*PS-column block [i | f | o | g], so one sigmoid covers the
            # whole active range (g-gate weights are pre-scaled by 2 so
            # tanh(x) = 2*sigmoid(2x)-1 applies).  Scans must activate /
            # deactivate so the active set is a contiguous range of slots.
            # tgc[:, a] holds [g-tilde | c] adjacent so one paired multiply
            # produces [i*g, f*c].
            # ----------------------------------------------------------------

            def make_group(name, scans, pool, psum_pool, ptag, shared_hseq=None):
                """shared_hseq: [128, slots+G*C-ish, G, 2, NL] mega-tile whose
                per-scan views are shifted by slot*C so concurrent scans
                (which trail each other by one chunk) write the same physical
                slot -> one merged h-write per step."""
                G = len(scans)
                g = dict(
                    name=name, scans=scans, G=G, ptag=ptag,
                    cst=pool.tile([128, G, PS], FP, tag=f"cst_{name}",
                                  name=f"cst_{name}"),
                    hq=shared_hseq,
                    psum=psum_pool,
                )
                for a, sc in enumerate(scans):
                    sc["grp"] = g
                    sc["slot"] = a
                return g

            def gp_alloc(g, actset):
                lo = min(s["slot"] for s in actset)
                hi = max(s["slot"] for s in actset)
                assert [s["slot"] for s in actset] == list(range(lo, hi + 1))
                # full psum bank: start=True lazily zeroes the whole 2KB
                # zero-region, so each step's accumulation chain owns a bank;
                # ptag (pipeline position) keys the pool slot so groups that
                # never run concurrently share banks
                gp = g["psum"].tile([128, 512], FP,
                                    tag=f"gp_{g['ptag']}", bufs=2,
                                    name=f"gp_{g['ptag']}")
                return gp, lo, hi

            def emit_copyin(g, gp, sc, i, first):
                """Seed psum accumulation regions with x-tilde for local
                step i (two matmuls: ifo block, g block).  Only the first
                matmul of the step's bank-wide chain sets start=True."""
                a = sc["slot"]
                lt = i if sc["fwd"] else (C - 1) - i
                if sc.get("bt"):
                    xs_ifo = sc["xsb"][:, 0:6, :, ds(lt, 1)]
                    xs_g = sc["xsb"][:, 6:8, :, ds(lt, 1)]
                else:
                    xs_ifo = sc["xsb"][:, 0:6, ds(lt, 1), :]
                    xs_g = sc["xsb"][:, 6:8, ds(lt, 1), :]
                c0 = a * 4 * PS
                nc.tensor.matmul(gp[:, c0:c0 + 3 * PS], ident, xs_ifo,
                                 start=first, stop=False)
                nc.tensor.matmul(gp[:, c0 + 3 * PS:c0 + 4 * PS], ident, xs_g,
                                 start=False, stop=False)

            def emit_whh(g, gp, sc, i, last):
                """16 recurrent matmuls for scan sc local step i; the last
                matmul of the bank-wide chain sets stop=True."""
                a = sc["slot"]
                t0 = sc["cur_chunk"] * C
                rslot = t0 + i if sc["fwd"] else sc["S"] - t0 - i
                whh, wbase = sc["whh"]
                c0 = a * 4 * PS
                for kk in range(2):
                    rh = sc["hseq"][:, ds(rslot, 1), kk, :]
                    for jn in range(8):
                        dst = gp[:, c0 + jn * NL:c0 + (jn + 1) * NL]
                        nc.tensor.matmul(
                            dst, whh[:, wbase + kk * 8 + jn, :], rh,
                            start=False, stop=(last and kk == 1 and jn == 7))

            def emit_stage(g, gp, lo, hi, i, stage):
                """Pipeline stages of the merged elementwise tail over the
                active slots lo..hi.
                  2: sigmoid over the whole gate bank  [Act]
                  3: g-tilde fixup, f*c, i*g, c add  [DVE]
                  4: tanh(c)  [Act]
                  5: h = o * tanh(c) writes  [DVE]"""
                cst = g["cst"]
                if stage == 2:
                    sfo = stash.tile([128, g["G"], 4, PS], FP,
                                     tag=f"sfo_{g['name']}")
                    g["sfo"] = sfo
                    nc.scalar.activation(
                        sfo[:, lo:hi + 1], gp[:, lo * 4 * PS:(hi + 1) * 4 * PS],
                        A.Sigmoid)
                elif stage == 3:
                    sfo = g["sfo"]
                    gt = stash.tile([128, g["G"], PS], FP, tag=f"gt_{g['name']}")
                    t1 = stash.tile([128, g["G"], PS], FP, tag=f"t1_{g['name']}")
                    t2 = stash.tile([128, g["G"], PS], FP, tag=f"t2_{g['name']}")
                    g["gt"] = gt
                    # g-tilde = 2*sigmoid(2x) - 1  (weights pre-scaled by 2);
                    # t1 = f*c right behind it reads only sig output, so it
                    # issues without waiting for the fixup's writeback
                    nc.vector.tensor_scalar(
                        gt[:, lo:hi + 1], sfo[:, lo:hi + 1, 3, :],
                        2.0, 1.0, mybir.AluOpType.mult, mybir.AluOpType.subtract)
                    nc.vector.tensor_mul(t1[:, lo:hi + 1],
                                         sfo[:, lo:hi + 1, 1, :], cst[:, lo:hi + 1])
                    nc.vector.tensor_mul(t2[:, lo:hi + 1],
                                         sfo[:, lo:hi + 1, 0, :], gt[:, lo:hi + 1])
                    nc.vector.tensor_add(cst[:, lo:hi + 1],
                                         t1[:, lo:hi + 1], t2[:, lo:hi + 1])
                elif stage == 4:
                    tcl = stash.tile([128, g["G"], PS], FP, tag=f"tc_{g['name']}")
                    g["tcl"] = tcl
                    nc.scalar.activation(tcl[:, lo:hi + 1],
                                         cst[:, lo:hi + 1], A.Tanh)
                else:
                    tcl = g["tcl"]
                    sfo = g["sfo"]
                    if g["hq"] is not None:
                        # shared-slot layout: every active scan writes the
                        # same physical slot -> one merged multiply
                        sc0 = g["scans"][lo]
                        t0 = sc0["cur_chunk"] * C
                        ps = t0 + i + 1 + lo * C
                        hw = g["hq"][:, ds(ps, 1), lo:hi + 1, :, :]
                        nc.vector.tensor_mul(hw, sfo[:, lo:hi + 1, 2, :],
                                             tcl[:, lo:hi + 1])
                    else:
                        for sc in g["scans"]:
                            a = sc["slot"]
                            if not (lo <= a <= hi):
                                continue
                            t0 = sc["cur_chunk"] * C
                            wslot = (t0 + i + 1) if sc["fwd"] else sc["S"] - 1 - t0 - i
                            hw = sc["hseq"][:, ds(wslot, 1), :, :]
                            nc.vector.tensor_mul(hw, sfo[:, a, 2, :], tcl[:, a, :])

            def run_phase(groups_active, pre=None, urgent=None):
                """groups_active: list of (group, actset); actset scans have
                cur_chunk set.  Emits C fully-unrolled steps, software-
                pipelining the groups half a step apart: group n+1's stages
                trail group n's so each engine's FIFO order matches the
                offset execution schedule (stage-5 of the trailing group is
                deferred one iteration).  pre: (hint, thunk)s for the next
                phase's x-tilde work, spread across iterations; "a"-hinted
                thunks emit right after the lead group's sigmoid (the Act
                engine's idle window), "d"-hinted ones after its h-write
                (the DVE idle window)."""
                pre = pre or []
                urgent = urgent or []
                sched_a, sched_d = {}, {}
                # urgent: current phase's remaining x-tilde ranges (consumed
                # from step XSPL[1] on) — densely packed into the first
                # iterations so they don't queue ahead of the first steps
                # 2/iteration so all are emitted before the pre-issued
                # copyin of step XSPL[1] (emitted at iteration XSPL[1]-1)
                for hint, sched in (("a", sched_a), ("d", sched_d)):
                    for t, th in enumerate(t for h, t in urgent if h == hint):
                        sched.setdefault(t // 2, []).append(th)
                pa = [t for h, t in pre if h == "a"]
                pd = [t for h, t in pre if h == "d"]
                for lst, sched in ((pa, sched_a), (pd, sched_d)):
                    if lst:
                        step = max(1, (C - 12) // len(lst))
                        for t, th in enumerate(lst):
                            sched.setdefault(min(4 + t * step, C - 2), []).append(th)
                ga = []
                for g, actset in groups_active:
                    gp, lo, hi = gp_alloc(g, actset)
                    for si, sc in enumerate(actset):
                        emit_copyin(g, gp, sc, 0, first=(si == 0))
                    ga.append([g, actset, gp, lo, hi, None])

                def s1(e, i):
                    g, actset, gp, lo, hi, _ = e
                    for si, sc in enumerate(actset):
                        emit_whh(g, gp, sc, i, last=(si == len(actset) - 1))
                    if i + 1 < C:
                        gpn, _, _ = gp_alloc(g, actset)
                        for si, sc in enumerate(actset):
                            emit_copyin(g, gpn, sc, i + 1, first=(si == 0))
                        e[5] = gpn

                if len(ga) == 1:
                    e = ga[0]
                    for i in range(C):
                        s1(e, i)
                        emit_stage(e[0], e[2], e[3], e[4], i, 2)
                        for th in sched_a.get(i, ()):
                            th()
                        emit_stage(e[0], e[2], e[3], e[4], i, 3)
                        emit_stage(e[0], e[2], e[3], e[4], i, 4)
                        emit_stage(e[0], e[2], e[3], e[4], i, 5)
                        for th in sched_d.get(i, ()):
                            th()
                        e[2] = e[5]
                else:
                    a, b = ga[0], ga[1]
                    for i in range(C):
                        s1(a, i)
                        emit_stage(a[0], a[2], a[3], a[4], i, 2)
                        for th in sched_a.get(i, ()):
                            th()
                        if i > 0:
                            emit_stage(b[0], b[2], b[3], b[4], i - 1, 5)
                            b[2] = b[5]
                        emit_stage(a[0], a[2], a[3], a[4], i, 3)
                        emit_stage(a[0], a[2], a[3], a[4], i, 4)
                        s1(b, i)
                        emit_stage(a[0], a[2], a[3], a[4], i, 5)
                        a[2] = a[5]
                        for th in sched_d.get(i, ()):
                            th()
                        emit_stage(b[0], b[2], b[3], b[4], i, 2)
                        emit_stage(b[0], b[2], b[3], b[4], i, 3)
                        emit_stage(b[0], b[2], b[3], b[4], i, 4)
                    emit_stage(b[0], b[2], b[3], b[4], C - 1, 5)

            # ---------- x-tilde chunk boundary (batched input projection) ----
            # Split into step ranges so the first steps of a phase unblock
            # after a small fraction of the projection work.  Range 0 packs
            # all 8 j-chunks into one psum bank (single accumulation chain,
            # single copy); later ranges go per-j.  Backward scans consume
            # the chunk back-to-front, so their ranges flip in time.
            XSPL = [0, 8, 24, C] if C > 32 else [0, 8, C]

            def _wih_tile(sc, kk, j):
                return (sc["wih"][0][:, sc["wih"][1] + kk * 8 + j, :]
                        if sc["wih"][2] else sc["wih"][0][:, j, :])

            def emit_xtilde_part(sc, k, r, j0=0, j1=8, xsb=None):
                """Emit the x-tilde projection for chunk k, step range r,
                j-chunks [j0, j1), into xsb (bound at thunk build time —
                sc['xsb_next'] may already point at the following chunk's
                buffer when a deferred thunk runs)."""
                nk = len(sc["xsrc"])
                s0, s1 = XSPL[r], XSPL[r + 1]
                a0, a1 = (s0, s1) if sc["fwd"] else (C - s1, C - s0)
                sub = a1 - a0
                bt = sc.get("bt")
                if xsb is None:
                    xsb = sc["xsb_next"]
                if r == 0:
                    shape = [128, 8, NL, sub] if bt else [128, 8, sub, NL]
                    xpa = psum_x.tile(shape, FP, tag="xpa", bufs=1, name="xpa")
                    for j in range(8):
                        for kk in range(nk):
                            rhs = sc["xsrc"][kk](k, a0, a1)
                            nc.tensor.matmul(xpa[:, j], _wih_tile(sc, kk, j), rhs,
                                             start=(j == 0 and kk == 0),
                                             stop=(not sc["brow"] and j == 7
                                                   and kk == nk - 1))
                        if sc["brow"]:
                            nc.tensor.matmul(
                                xpa[:, j], sc["brow"][0][:, sc["brow"][1] + j, :],
                                ones[:, 0:sub * NL], start=False, stop=(j == 7))
                    dst = (xsb[:, :, :, a0:a1] if bt else xsb[:, :, a0:a1, :])
                    nc.vector.tensor_copy(dst, xpa)
                else:
                    for j in range(j0, j1):
                        # full bank regardless of CB: an accumulation chain's
                        # start=True lazily zeroes its whole 2KB zero-region
                        xp = psum_x.tile([128, 512], FP, tag="xp")
                        for kk in range(nk):
                            rhs = sc["xsrc"][kk](k, a0, a1)
                            nc.tensor.matmul(xp[:, 0:sub * NL],
                                             _wih_tile(sc, kk, j), rhs,
                                             start=(kk == 0),
                                             stop=False if sc["brow"] else (kk == nk - 1))
                        if sc["brow"]:
                            nc.tensor.matmul(xp[:, 0:sub * NL],
                                             sc["brow"][0][:, sc["brow"][1] + j, :],
                                             ones[:, 0:sub * NL], start=False, stop=True)
                        dst = (xsb[:, j, :, a0:a1] if bt
                               else xsb[:, j, a0:a1, :])
                        if j % 2 == 0:
                            nc.scalar.copy(dst, xp[:, 0:sub * NL])
                        else:
                            nc.vector.tensor_copy(dst, xp[:, 0:sub * NL])

            def xtilde_thunks(sc, k):
                """(engine_hint, thunk) list for chunk k's projection, each
                small enough (~1 psum->sbuf copy) to hide in a step's idle
                window on that engine.  Allocates the destination buffer
                eagerly so thunk emission order within the phase is free."""
                xsb = sc["pool"].tile(
                    [128, 8, NL, C] if sc.get("bt") else [128, 8, C, NL],
                    BF, tag=f"x_{sc['tag']}", bufs=sc.get("xbufs", 1),
                    name=f"x_{sc['tag']}")
                sc["xsb_next"] = xsb
                th = [(sc.get("h0", "d"),
                       lambda sc=sc, k=k, xsb=xsb:
                       emit_xtilde_part(sc, k, 0, xsb=xsb))]
                for r in range(1, len(XSPL) - 1):
                    for j in range(8):
                        th.append(("a" if j % 2 == 0 else "d",
                                   lambda sc=sc, k=k, r=r, j=j, xsb=xsb:
                                   emit_xtilde_part(sc, k, r, j, j + 1, xsb=xsb)))
                return th

            def xtilde_commit(sc, k):
                sc["cur_chunk"] = k
                sc["xsb"] = sc["xsb_next"]

            # =======================================================
            # ENCODER
            # =======================================================
            with ExitStack() as ectx:
                epool = ectx.enter_context(tc.tile_pool(name="enc", bufs=1))
                psg_e = ectx.enter_context(
                    tc.tile_pool(name="psge", bufs=1, space="PSUM"))
                ew_ih = epool.tile([128, 128, 128], BF)
                ew_hh = epool.tile([128, 64, 128], BF)
                ew_b = epool.tile([1, 32, 128], BF)
                nc.sync.dma_start(out=ew_ih, in_=d_ewih)
                nc.sync.dma_start(out=ew_hh, in_=d_ewhh)
                nc.sync.dma_start(out=ew_b, in_=d_eb)

                eo_bf = epool.tile([128, 4, S, NL], BF)
                for kk in range(4):
                    nc.sync.dma_start(out=eo_bf[:, kk], in_=d_encrhs[kk])

                escan = {}
                for (l, d) in [(0, 0), (0, 1), (1, 0), (1, 1)]:
                    tag = f"e{l}{d}"
                    hseq = epool.tile([128, S + 1, 2, NL], BF, tag=f"hs_{tag}")
                    init_slot = 0 if d == 0 else S
                    nc.vector.memset(hseq[:, init_slot], 0.0)
                    widx = ((l * 2 + d) * 2) * 8
                    wxidx = ((l * 2 + d) * 4) * 8
                    bidx = (l * 2 + d) * 8
                    if l == 0:
                        xsrc = []
                        for kk in range(4):
                            def f(k, a0, a1, kk=kk, d=d):
                                tr0 = k * C if d == 0 else S - (k + 1) * C
                                return eo_bf[:, kk, tr0 + a0:tr0 + a1, :]
                            xsrc.append(f)
                    else:
                        xsrc = []
                        for kk in range(4):
                            def f(k, a0, a1, kk=kk, d=d):
                                tr0 = k * C if d == 0 else S - (k + 1) * C
                                if kk < 2:
                                    return escan["e00"]["hseq"][:, tr0 + 1 + a0:tr0 + 1 + a1, kk, :]
                                else:
                                    return escan["e01"]["hseq"][:, tr0 + a0:tr0 + a1, kk - 2, :]
                            xsrc.append(f)
                    escan[tag] = dict(
                        tag=tag, fwd=(d == 0), S=S, hseq=hseq, pool=epool,
                        whh=(ew_hh, widx), wih=(ew_ih, wxidx, True),
                        brow=(ew_b, bidx), xsrc=xsrc, cur_chunk=0, xbufs=2)

                # one singleton group per scan; fwd/bwd pipeline as A/B
                egroups = {}
                for (l, d) in [(0, 0), (0, 1), (1, 0), (1, 1)]:
                    tag = f"e{l}{d}"
                    egroups[tag] = make_group(tag + "g", [escan[tag]], epool,
                                              psg_e, ptag="AB"[d])
                    nc.vector.memset(egroups[tag]["cst"], 0.0)

                # layer-l chunk-(k+1) projections depend only on eo_bf (l=0)
                # or the fully-written L0 hseqs (l=1), so they pre-emit
                # inside chunk k's steps; only each layer's first chunk pays
                # the projection latency up front.
                for l in range(2):
                    gf, gb = egroups[f"e{l}0"], egroups[f"e{l}1"]
                    scans = (gf["scans"][0], gb["scans"][0])
                    for k in range(SC):
                        urgent = []
                        if k == 0:
                            for _, th in [t for sc in scans
                                          for t in xtilde_thunks(sc, 0)]:
                                th()
                        for sc in scans:
                            xtilde_commit(sc, k)
                        pre = []
                        if k + 1 < SC:
                            pre = [t for sc in scans
                                   for t in xtilde_thunks(sc, k + 1)]
                        run_phase([(gf, gf["scans"]), (gb, gb["scans"])],
                                  pre=pre, urgent=urgent)

                # copy finals into persistent init tiles
                fin = ["e00", "e01", "e10", "e11"]
                for li, tag in enumerate(fin):
                    slot = S if tag.endswith("0") else 0
                    nc.vector.tensor_copy(hinit[li], escan[tag]["hseq"][:, slot])
                    nc.vector.tensor_copy(cinit[li], egroups[tag]["cst"][:, 0])

            # =======================================================
            # DECODER (4-layer chunk-lagged wavefront, 2 groups of 2)
            # =======================================================
            with ExitStack() as dctx:
                dpool = dctx.enter_context(tc.tile_pool(name="dec", bufs=1))
                psg_d = dctx.enter_context(
                    tc.tile_pool(name="psgd", bufs=1, space="PSUM"))
                dw_ih0 = dpool.tile([97, 8, 128], BF)
                dw_ih = dpool.tile([128, 48, 128], BF)
                dw_b = dpool.tile([1, 24, 128], BF)
                dw_hh = dpool.tile([128, 64, 128], BF)
                fw = dpool.tile([128, 2, NMEL], BF)
                fb = dpool.tile([1, NMEL], BF)
                nc.sync.dma_start(out=dw_ih0, in_=d_dwih0)
                nc.sync.dma_start(out=dw_ih, in_=d_dwih)
                nc.sync.dma_start(out=dw_b, in_=d_db)
                nc.sync.dma_start(out=dw_hh, in_=d_dwhh)
                nc.sync.dma_start(out=fw, in_=d_fcw)
                nc.sync.dma_start(out=fb, in_=d_fcb)

                teach = dpool.tile([97, NL, C], BF)
                nc.vector.memset(teach, 0.0)
                nc.vector.memset(teach[96:97], 1.0)
                mst = dpool.tile([NMEL, NL, C], FP)
                icst = dpool.tile([NMEL, 1], FP)

                # shared hseq mega-tiles: layer 2g+1's view is shifted +C
                # slots so both group members write one physical slot/step
                hq_t = [dpool.tile([128, TD + 1 + C, 2, 2, NL], BF,
                                   tag=f"hq{gi}", name=f"hq{gi}")
                        for gi in range(2)]

                dscan = []
                for l in range(4):
                    tag = f"d{l}"
                    a = l % 2
                    hseq = hq_t[l // 2][:, a * C:a * C + TD + 1, a, :, :]
                    nc.vector.tensor_copy(hseq[:, 0], hinit[l])
                    if l == 0:
                        wih = (dw_ih0, 0, False)
                        brow = None
                        xsrc = [lambda k, a0, a1: teach[:, :, a0:a1]]
                    else:
                        wih = (dw_ih, (l - 1) * 16, True)
                        brow = (dw_b, (l - 1) * 8)
                        xsrc = []
                        for kk in range(2):
                            def f(k, a0, a1, kk=kk, l=l):
                                t0 = k * C
                                return dscan[l - 1]["hseq"][:, t0 + 1 + a0:t0 + 1 + a1, kk, :]
                            xsrc.append(f)
                    dscan.append(dict(
                        tag=tag, fwd=True, S=TD, hseq=hseq, pool=dpool,
                        whh=(dw_hh, l * 16), wih=wih, brow=brow, xsrc=xsrc,
                        bt=(l == 0), cur_chunk=0, xbufs=2 if l == 0 else 1))

                dgroups = [
                    make_group("d0", [dscan[0], dscan[1]], dpool, psg_d, "A",
                               shared_hseq=hq_t[0]),
                    make_group("d1", [dscan[2], dscan[3]], dpool, psg_d, "B",
                               shared_hseq=hq_t[1]),
                ]
                for gi, g in enumerate(dgroups):
                    for a in range(2):
                        nc.vector.tensor_copy(g["cst"][:, a], cinit[gi * 2 + a])

                mels_r = d_mels.rearrange("n c t -> c n t")  # [80, NL, T]

                def fill_teacher(k):
                    t0 = k * C
                    if k == 0:
                        nc.sync.dma_start(out=icst, in_=d_ictx.rearrange("(c o) -> c o", o=1))
                        nc.vector.tensor_copy(
                            mst[:, :, 0], icst.to_broadcast((NMEL, NL)))
                        nc.sync.dma_start(
                            out=mst[:, :, 1:], in_=mels_r[:, :, 0:C - 1])
                    else:
                        nc.sync.dma_start(
                            out=mst, in_=mels_r[:, :, t0 - 1:t0 + C - 1])
                    nc.vector.tensor_copy(teach[0:NMEL, :], mst)

                def emit_fc(k):
                    t0 = k * C
                    fpt = psum_x.tile([NMEL, 512], FP, tag="xp", name=f"fcp{k}")
                    fp = fpt[:, 0:CB]
                    for kk in range(2):
                        nc.tensor.matmul(
                            fp, fw[:, kk, :],
                            dscan[3]["hseq"][:, t0 + 1:t0 + C + 1, kk, :],
                            start=(kk == 0), stop=False)
                    nc.tensor.matmul(fp, fb, ones, start=False, stop=True)
                    fst = stash.tile([NMEL, NL, C], FP, tag="fst", name=f"fst{k}")
                    nc.scalar.copy(fst, fp.rearrange("p (t b) -> p b t", b=NL))
                    if k == DC - 1:
                        nc.vector.memset(fst[:, :, C - 1:C], 0.0)
                    nc.sync.dma_start(out=out_r[:, :, t0:t0 + C], in_=fst)

                # wavefront phases; layer-0 chunk p+1 projections (teacher:
                # input-only) pre-emit inside phase p; layers 1-3 need the
                # previous layer's chunk, so theirs stay at the boundary
                for p in range(DC + 3):
                    active = [l for l in range(4) if 0 <= p - l < DC]
                    urgent = []
                    if p == 0:
                        fill_teacher(0)
                        for _, th in xtilde_thunks(dscan[0], 0):
                            th()
                    for l in active:
                        if l >= 1:
                            for _, th in xtilde_thunks(dscan[l], p - l):
                                th()
                    for l in active:
                        xtilde_commit(dscan[l], p - l)
                    pre = []
                    if p + 1 < DC:
                        pre = ([("d", lambda p=p: fill_teacher(p + 1))]
                               + xtilde_thunks(dscan[0], p + 1))
                    groups_active = []
                    for gi, g in enumerate(dgroups):
                        actset = [dscan[l] for l in active if l // 2 == gi]
                        if actset:
                            groups_active.append((g, actset))
                    run_phase(groups_active, pre=pre, urgent=urgent)
                    if 0 <= p - 4 < DC:
                        emit_fc(p - 4)

                fc_done = {p - 4 for p in range(DC + 3) if 0 <= p - 4 < DC}
                for k in range(DC):
                    if k not in fc_done:
                        emit_fc(k)

    nc.compile()
    return nc


def _host_prep(inputs):
    """Slice batch across cores + pre-tile/cast weights (gate order i,f,o,g).
    Returns in_maps."""
    bf16 = ml_dtypes.bfloat16

    def tiles_T(w, kchunks, jchunks):
        # w: [4H, D] fp32 -> [k, j, 128, 128] bf16 tiles of w.T, j in
        # GATE_PERM order (i,f,o,g); g-gate tiles (j >= 6) scaled by 2 so
        # tanh(x) = 2*sigmoid(2x)-1 turns the whole gate bank into one
        # sigmoid (scale in fp32, before the bf16 cast)
        wT = np.ascontiguousarray(w.T.astype(np.float32))  # [D, 4H]
        out = np.zeros((kchunks, jchunks, 128, 128), bf16)
        for k in range(kchunks):
            for j in range(jchunks):
                jo = GATE_PERM[j]
                t = wT[k * 128:(k + 1) * 128, jo * 128:(jo + 1) * 128]
                out[k, j] = ((t * 2.0) if j >= 6 else t).astype(bf16)
        return out

    def perm_b(b):
        # b: [4H] -> [8, 128] rows in GATE_PERM order, g rows scaled by 2
        r = b.reshape(8, 128).astype(np.float32)[GATE_PERM].copy()
        r[6:8] *= 2.0
        return r

    enc_Wih = np.asarray(inputs["enc_Wih"], np.float32)
    enc_Whh = np.asarray(inputs["enc_Whh"], np.float32)
    enc_b = np.asarray(inputs["enc_b"], np.float32)
    dec_Wih0 = np.asarray(inputs["dec_Wih0"], np.float32)
    dec_Wih = np.asarray(inputs["dec_Wih"], np.float32)
    dec_Whh = np.asarray(inputs["dec_Whh"], np.float32)
    dec_b = np.asarray(inputs["dec_b"], np.float32)
    fc_W = np.asarray(inputs["fc_W"], np.float32)
    fc_b = np.asarray(inputs["fc_b"], np.float32)
    ictx = np.asarray(inputs["init_ctx"], np.float32).reshape(-1)

    ewih = np.zeros((2, 2, 4, 8, 128, 128), bf16)
    ewhh = np.zeros((2, 2, 2, 8, 128, 128), bf16)
    eb = np.zeros((2, 2, 8, 128), bf16)
    for l in range(2):
        for d in range(2):
            ewih[l, d] = tiles_T(enc_Wih[l, d], 4, 8)
            ewhh[l, d] = tiles_T(enc_Whh[l, d], 2, 8)
            eb[l, d] = perm_b(enc_b[l, d]).astype(bf16)

    dwih0 = np.zeros((97, 8, 128), bf16)
    w0T = dec_Wih0.T.astype(np.float32)  # [80, 1024]
    db0 = perm_b(dec_b[0])
    for j in range(8):
        jo = GATE_PERM[j]
        t = w0T[:, jo * 128:(jo + 1) * 128]
        dwih0[0:80, j] = ((t * 2.0) if j >= 6 else t).astype(bf16)
        dwih0[96, j] = db0[j].astype(bf16)

    dwih = np.zeros((3, 2, 8, 128, 128), bf16)
    db = np.zeros((3, 8, 128), bf16)
    for l in range(3):
        dwih[l] = tiles_T(dec_Wih[l], 2, 8)
        db[l] = perm_b(dec_b[l + 1]).astype(bf16)
    dwhh = np.zeros((4, 2, 8, 128, 128), bf16)
    for l in range(4):
        dwhh[l] = tiles_T(dec_Whh[l], 2, 8)

    fcw = np.zeros((2, 128, NMEL), bf16)
    fWT = fc_W.T.astype(bf16)  # [256, 80]
    fcw[0] = fWT[0:128]
    fcw[1] = fWT[128:256]

    encout = np.asarray(inputs["encoder_outputs"], np.float32)
    mels = np.asarray(inputs["mels"], np.float32)
    N = encout.shape[0]
    nb = N // NCORES

    base = {
        "ictx": ictx,
        "ewih": np.ascontiguousarray(ewih.reshape(128, 128, 128).transpose(1, 0, 2)),
        "ewhh": np.ascontiguousarray(ewhh.reshape(64, 128, 128).transpose(1, 0, 2)),
        "eb": np.ascontiguousarray(eb.reshape(1, 32, 128)),
        "dwih0": dwih0,
        "dwih": np.ascontiguousarray(dwih.reshape(48, 128, 128).transpose(1, 0, 2)),
        "db": np.ascontiguousarray(db.reshape(1, 24, 128)),
        "dwhh": np.ascontiguousarray(dwhh.reshape(64, 128, 128).transpose(1, 0, 2)),
        "fcw": np.ascontiguousarray(fcw.transpose(1, 0, 2)),
        "fcb": fc_b.astype(bf16).reshape(1, NMEL),
    }
    S = encout.shape[1]
    in_maps = []
    for cid in range(NCORES):
        m = dict(base)
        eo = encout[cid * nb:(cid + 1) * nb]  # [nb, S, 512]
        m["encrhs"] = np.ascontiguousarray(
            eo.transpose(2, 1, 0).reshape(4, 128, S, nb).astype(bf16))
        m["mels"] = np.ascontiguousarray(mels[cid * nb:(cid + 1) * nb])
        in_maps.append(m)
    return in_maps


def kernel(encoder_outputs, mels, text_lengths, output_lengths,
           enc_Wih, enc_Whh, enc_b, dec_Wih0, dec_Wih, dec_Whh, dec_b,
           fc_W, fc_b, init_ctx):
    from concourse import bass_utils

    inputs = dict(encoder_outputs=encoder_outputs, mels=mels,
                  enc_Wih=enc_Wih, enc_Whh=enc_Whh, enc_b=enc_b,
                  dec_Wih0=dec_Wih0, dec_Wih=dec_Wih, dec_Whh=dec_Whh,
                  dec_b=dec_b, fc_W=fc_W, fc_b=fc_b, init_ctx=init_ctx)
    N, S, _ = np.asarray(encoder_outputs).shape
    T = np.asarray(mels).shape[2]
    key = (S, T)
    if key not in _prog_cache:
        _prog_cache[key] = _build_program(S, T)
    nc = _prog_cache[key]
    in_maps = _host_prep(inputs)
    res = bass_utils.run_bass_kernel_spmd(nc, in_maps, core_ids=list(range(NCORES)))
    nb = N // NCORES
    out = np.zeros((N, NMEL, T), np.float32)
    for cid in range(NCORES):
        out[cid * nb:(cid + 1) * nb] = res.results[cid]["out"]
    return (out,)


# revision 73
# speedup vs baseline: 1.0040x; 1.0040x over previous
"""Trainium2 Bass kernel for nn_Decoder (Tacotron-style LSTM encoder/decoder).

Architecture (8 NeuronCores, data-parallel over batch N=64 -> 8/core):
  - Transposed ("World B") layout: hidden dim on SBUF partitions, (chunk,batch)
    on the free dim, so the h produced by the elementwise tail is directly the
    next step's matmul rhs (no per-step transposes).
  - Teacher forcing / layer chunking: input-side projections are batched into
    large matmuls per 64-step chunk; only h @ Whh.T stays per-step.
  - FULLY UNROLLED instruction stream (no hardware loops): loop-register
    updates (FusedRegOps, 96ns each on the PE sequencer) dominated the looped
    version; immediate addressing removes them entirely.
  - Gate order re-tiled host-side to i,f,o,g so one sigmoid instruction covers
    i/f/o and one tanh covers g.
  - Concurrent scans (encoder fwd+bwd; decoder wavefront layers grouped in
    pairs) share one PSUM gate bank so the elementwise tail is emitted once
    per group, not once per scan.
  - Weights / h / x-projections in bf16, cell state c and PSUM in fp32.
"""

import numpy as np
import ml_dtypes

H = 256
NMEL = 80
D_ENC = 512
NCORES = 8
NL = 8          # batch per core
C = 32          # chunk (time) size (smaller chunk = shorter wavefront fill)

# new gate order: i,f,o,g (old torch order i,f,g,o); 8 j-chunks of 128
GATE_PERM = [0, 1, 2, 3, 6, 7, 4, 5]

_prog_cache = {}


def _build_program(S, T):
    """Build the Bass program for full sequence length S (encoder) and T
    (mels length; decoder runs TD = T padded steps). Returns nc."""
    import concourse.bass as bass
    import concourse.mybir as mybir
    import concourse.tile as tile
    from concourse import bacc
    from concourse.bass import ds
    from concourse.masks import make_identity
    from contextlib import ExitStack

    BF = mybir.dt.bfloat16
    FP = mybir.dt.float32
    A = mybir.ActivationFunctionType

    TD = T
    assert S % C == 0 and TD % C == 0
    SC = S // C
    DC = TD // C

    nc = bacc.Bacc("TRN2", target_bir_lowering=False, debug=False,
                   num_devices=NCORES)

    # ---------------- DRAM I/O ----------------
    d_encrhs = nc.dram_tensor("encrhs", [4, 128, S, NL], BF, kind="ExternalInput").ap()
    d_mels = nc.dram_tensor("mels", [NL, NMEL, T], FP, kind="ExternalInput").ap()
    d_ictx = nc.dram_tensor("ictx", [NMEL], FP, kind="ExternalInput").ap()
    d_ewih = nc.dram_tensor("ewih", [128, 128, 128], BF, kind="ExternalInput").ap()
    d_ewhh = nc.dram_tensor("ewhh", [128, 64, 128], BF, kind="ExternalInput").ap()
    d_eb = nc.dram_tensor("eb", [1, 32, 128], BF, kind="ExternalInput").ap()
    d_dwih0 = nc.dram_tensor("dwih0", [97, 8, 128], BF, kind="ExternalInput").ap()
    d_dwih = nc.dram_tensor("dwih", [128, 48, 128], BF, kind="ExternalInput").ap()
    d_db = nc.dram_tensor("db", [1, 24, 128], BF, kind="ExternalInput").ap()
    d_dwhh = nc.dram_tensor("dwhh", [128, 64, 128], BF, kind="ExternalInput").ap()
    d_fcw = nc.dram_tensor("fcw", [128, 2, NMEL], BF, kind="ExternalInput").ap()
    d_fcb = nc.dram_tensor("fcb", [1, NMEL], BF, kind="ExternalInput").ap()
    d_out = nc.dram_tensor("out", [NL, NMEL, T], FP, kind="ExternalOutput").ap()
    out_r = d_out.rearrange("n c t -> c n t")  # [80, NL, T]

    CB = C * NL  # tokens per chunk = 512
    PS = 2 * NL  # per-scan gate-chunk block (kk-major x batch) = 16

    with tile.TileContext(nc) as tc:
        with ExitStack() as ctx:
            persist = ctx.enter_context(tc.tile_pool(name="persist", bufs=1))
            psum_x = ctx.enter_context(
                tc.tile_pool(name="psx", bufs=2, space="PSUM"))
            stash = ctx.enter_context(tc.tile_pool(name="stash", bufs=2))

            ident = persist.tile([128, 128], BF)
            make_identity(nc, ident)
            ones = persist.tile([1, CB], BF)
            nc.vector.memset(ones, 1.0)

            # decoder init states copied out of encoder scope
            hinit = [persist.tile([128, PS], BF, tag=f"hi{l}", name=f"hinit{l}")
                     for l in range(4)]
            cinit = [persist.tile([128, PS], FP, tag=f"ci{l}", name=f"cinit{l}")
                     for l in range(4)]

            # ----------------------------------------------------------------
            # A "group" merges the elementwise tail of several concurrent
            # scans.  Group PSUM layout: per scan slot a, a contiguous
            # 4*PS-column block [i | f | o | g], so one sigmoid covers the
            # whole active range (g-gate weights are pre-scaled by 2 so
            # tanh(x) = 2*sigmoid(2x)-1 applies).  Scans must activate /
            # deactivate so the active set is a contiguous range of slots.
            # tgc[:, a] holds [g-tilde | c] adjacent so one paired multiply
            # produces [i*g, f*c].
            # ----------------------------------------------------------------

            def make_group(name, scans, pool, psum_pool, ptag, shared_hseq=None):
                """shared_hseq: [128, slots+G*C-ish, G, 2, NL] mega-tile whose
                per-scan views are shifted by slot*C so concurrent scans
                (which trail each other by one chunk) write the same physical
                slot -> one merged h-write per step."""
                G = len(scans)
                g = dict(
                    name=name, scans=scans, G=G, ptag=ptag,
                    cst=pool.tile([128, G, PS], FP, tag=f"cst_{name}",
                                  name=f"cst_{name}"),
                    hq=shared_hseq,
                    psum=psum_pool,
                )
                for a, sc in enumerate(scans):
                    sc["grp"] = g
                    sc["slot"] = a
                return g

            def gp_alloc(g, actset):
                lo = min(s["slot"] for s in actset)
                hi = max(s["slot"] for s in actset)
                assert [s["slot"] for s in actset] == list(range(lo, hi + 1))
                # full psum bank: start=True lazily zeroes the whole 2KB
                # zero-region, so each step's accumulation chain owns a bank;
                # ptag (pipeline position) keys the pool slot so groups that
                # never run concurrently share banks
                gp = g["psum"].tile([128, 512], FP,
                                    tag=f"gp_{g['ptag']}", bufs=2,
                                    name=f"gp_{g['ptag']}")
                return gp, lo, hi

            def emit_copyin(g, gp, sc, i, first):
                """Seed psum accumulation regions with x-tilde for local
                step i (two matmuls: ifo block, g block).  Only the first
                matmul of the step's bank-wide chain sets start=True."""
                a = sc["slot"]
                lt = i if sc["fwd"] else (C - 1) - i
                if sc.get("bt"):
                    xs_ifo = sc["xsb"][:, 0:6, :, ds(lt, 1)]
                    xs_g = sc["xsb"][:, 6:8, :, ds(lt, 1)]
                else:
                    xs_ifo = sc["xsb"][:, 0:6, ds(lt, 1), :]
                    xs_g = sc["xsb"][:, 6:8, ds(lt, 1), :]
                c0 = a * 4 * PS
                nc.tensor.matmul(gp[:, c0:c0 + 3 * PS], ident, xs_ifo,
                                 start=first, stop=False)
                nc.tensor.matmul(gp[:, c0 + 3 * PS:c0 + 4 * PS], ident, xs_g,
                                 start=False, stop=False)

            def emit_whh(g, gp, sc, i, last):
                """16 recurrent matmuls for scan sc local step i; the last
                matmul of the bank-wide chain sets stop=True."""
                a = sc["slot"]
                t0 = sc["cur_chunk"] * C
                rslot = t0 + i if sc["fwd"] else sc["S"] - t0 - i
                whh, wbase = sc["whh"]
                c0 = a * 4 * PS
                for kk in range(2):
                    rh = sc["hseq"][:, ds(rslot, 1), kk, :]
                    for jn in range(8):
                        dst = gp[:, c0 + jn * NL:c0 + (jn + 1) * NL]
                        nc.tensor.matmul(
                            dst, whh[:, wbase + kk * 8 + jn, :], rh,
                            start=False, stop=(last and kk == 1 and jn == 7))

            def emit_stage(g, gp, lo, hi, i, stage):
                """Pipeline stages of the merged elementwise tail over the
                active slots lo..hi.
                  2: sigmoid over the whole gate bank  [Act]
                  3: g-tilde fixup, f*c, i*g, c add  [DVE]
                  4: tanh(c)  [Act]
                  5: h = o * tanh(c) writes  [DVE]"""
                cst = g["cst"]
                if stage == 2:
                    sfo = stash.tile([128, g["G"], 4, PS], FP,
                                     tag=f"sfo_{g['name']}")
                    g["sfo"] = sfo
                    nc.scalar.activation(
                        sfo[:, lo:hi + 1], gp[:, lo * 4 * PS:(hi + 1) * 4 * PS],
                        A.Sigmoid)
                elif stage == 3:
                    sfo = g["sfo"]
                    gt = stash.tile([128, g["G"], PS], FP, tag=f"gt_{g['name']}")
                    t1 = stash.tile([128, g["G"], PS], FP, tag=f"t1_{g['name']}")
                    t2 = stash.tile([128, g["G"], PS], FP, tag=f"t2_{g['name']}")
                    g["gt"] = gt
                    # g-tilde = 2*sigmoid(2x) - 1  (weights pre-scaled by 2);
                    # t1 = f*c right behind it reads only sig output, so it
                    # issues without waiting for the fixup's writeback
                    nc.vector.tensor_scalar(
                        gt[:, lo:hi + 1], sfo[:, lo:hi + 1, 3, :],
                        2.0, 1.0, mybir.AluOpType.mult, mybir.AluOpType.subtract)
                    nc.vector.tensor_mul(t1[:, lo:hi + 1],
                                         sfo[:, lo:hi + 1, 1, :], cst[:, lo:hi + 1])
                    nc.vector.tensor_mul(t2[:, lo:hi + 1],
                                         sfo[:, lo:hi + 1, 0, :], gt[:, lo:hi + 1])
                    nc.vector.tensor_add(cst[:, lo:hi + 1],
                                         t1[:, lo:hi + 1], t2[:, lo:hi + 1])
                elif stage == 4:
                    tcl = stash.tile([128, g["G"], PS], FP, tag=f"tc_{g['name']}")
                    g["tcl"] = tcl
                    nc.scalar.activation(tcl[:, lo:hi + 1],
                                         cst[:, lo:hi + 1], A.Tanh)
                else:
                    tcl = g["tcl"]
                    sfo = g["sfo"]
                    if g["hq"] is not None:
                        # shared-slot layout: every active scan writes the
                        # same physical slot -> one merged multiply
                        sc0 = g["scans"][lo]
                        t0 = sc0["cur_chunk"] * C
                        ps = t0 + i + 1 + lo * C
                        hw = g["hq"][:, ds(ps, 1), lo:hi + 1, :, :]
                        nc.vector.tensor_mul(hw, sfo[:, lo:hi + 1, 2, :],
                                             tcl[:, lo:hi + 1])
                    else:
                        for sc in g["scans"]:
                            a = sc["slot"]
                            if not (lo <= a <= hi):
                                continue
                            t0 = sc["cur_chunk"] * C
                            wslot = (t0 + i + 1) if sc["fwd"] else sc["S"] - 1 - t0 - i
                            hw = sc["hseq"][:, ds(wslot, 1), :, :]
                            nc.vector.tensor_mul(hw, sfo[:, a, 2, :], tcl[:, a, :])

            def run_phase(groups_active, pre=None, urgent=None):
                """groups_active: list of (group, actset); actset scans have
                cur_chunk set.  Emits C fully-unrolled steps, software-
                pipelining the groups half a step apart: group n+1's stages
                trail group n's so each engine's FIFO order matches the
                offset execution schedule (stage-5 of the trailing group is
                deferred one iteration).  pre: (hint, thunk)s for the next
                phase's x-tilde work, spread across iterations; "a"-hinted
                thunks emit right after the lead group's sigmoid (the Act
                engine's idle window), "d"-hinted ones after its h-write
                (the DVE idle window)."""
                pre = pre or []
                urgent = urgent or []
                sched_a, sched_d = {}, {}
                # urgent: current phase's remaining x-tilde ranges (consumed
                # from step XSPL[1] on) — densely packed into the first
                # iterations so they don't queue ahead of the first steps
                # 2/iteration so all are emitted before the pre-issued
                # copyin of step XSPL[1] (emitted at iteration XSPL[1]-1)
                for hint, sched in (("a", sched_a), ("d", sched_d)):
                    for t, th in enumerate(t for h, t in urgent if h == hint):
                        sched.setdefault(t // 2, []).append(th)
                pa = [t for h, t in pre if h == "a"]
                pd = [t for h, t in pre if h == "d"]
                for lst, sched in ((pa, sched_a), (pd, sched_d)):
                    if lst:
                        step = max(1, (C - 12) // len(lst))
                        for t, th in enumerate(lst):
                            sched.setdefault(min(4 + t * step, C - 2), []).append(th)
                ga = []
                for g, actset in groups_active:
                    gp, lo, hi = gp_alloc(g, actset)
                    for si, sc in enumerate(actset):
                        emit_copyin(g, gp, sc, 0, first=(si == 0))
                    ga.append([g, actset, gp, lo, hi, None])

                def s1(e, i):
                    g, actset, gp, lo, hi, _ = e
                    for si, sc in enumerate(actset):
                        emit_whh(g, gp, sc, i, last=(si == len(actset) - 1))
                    if i + 1 < C:
                        gpn, _, _ = gp_alloc(g, actset)
                        for si, sc in enumerate(actset):
                            emit_copyin(g, gpn, sc, i + 1, first=(si == 0))
                        e[5] = gpn

                if len(ga) == 1:
                    e = ga[0]
                    for i in range(C):
                        s1(e, i)
                        emit_stage(e[0], e[2], e[3], e[4], i, 2)
                        for th in sched_a.get(i, ()):
                            th()
                        emit_stage(e[0], e[2], e[3], e[4], i, 3)
                        emit_stage(e[0], e[2], e[3], e[4], i, 4)
                        emit_stage(e[0], e[2], e[3], e[4], i, 5)
                        for th in sched_d.get(i, ()):
                            th()
                        e[2] = e[5]
                else:
                    a, b = ga[0], ga[1]
                    for i in range(C):
                        s1(a, i)
                        emit_stage(a[0], a[2], a[3], a[4], i, 2)
                        for th in sched_a.get(i, ()):
                            th()
                        if i > 0:
                            emit_stage(b[0], b[2], b[3], b[4], i - 1, 5)
                            b[2] = b[5]
                        emit_stage(a[0], a[2], a[3], a[4], i, 3)
                        emit_stage(a[0], a[2], a[3], a[4], i, 4)
                        s1(b, i)
                        emit_stage(a[0], a[2], a[3], a[4], i, 5)
                        a[2] = a[5]
                        for th in sched_d.get(i, ()):
                            th()
                        emit_stage(b[0], b[2], b[3], b[4], i, 2)
                        emit_stage(b[0], b[2], b[3], b[4], i, 3)
                        emit_stage(b[0], b[2], b[3], b[4], i, 4)
                    emit_stage(b[0], b[2], b[3], b[4], C - 1, 5)

            # ---------- x-tilde chunk boundary (batched input projection) ----
            # Split into step ranges so the first steps of a phase unblock
            # after a small fraction of the projection work.  Range 0 packs
            # all 8 j-chunks into one psum bank (single accumulation chain,
            # single copy); later ranges go per-j.  Backward scans consume
            # the chunk back-to-front, so their ranges flip in time.
            XSPL = [0, 8, 24, C] if C > 32 else [0, 4, C]

            def _wih_tile(sc, kk, j):
                return (sc["wih"][0][:, sc["wih"][1] + kk * 8 + j, :]
                        if sc["wih"][2] else sc["wih"][0][:, j, :])

            def emit_xtilde_part(sc, k, r, j0=0, j1=8, xsb=None):
                """Emit the x-tilde projection for chunk k, step range r,
                j-chunks [j0, j1), into xsb (bound at thunk build time —
                sc['xsb_next'] may already point at the following chunk's
                buffer when a deferred thunk runs)."""
                nk = len(sc["xsrc"])
                s0, s1 = XSPL[r], XSPL[r + 1]
                a0, a1 = (s0, s1) if sc["fwd"] else (C - s1, C - s0)
                sub = a1 - a0
                bt = sc.get("bt")
                if xsb is None:
                    xsb = sc["xsb_next"]
                if r == 0:
                    shape = [128, 8, NL, sub] if bt else [128, 8, sub, NL]
                    xpa = psum_x.tile(shape, FP, tag="xpa", bufs=1, name="xpa")
                    for j in range(8):
                        for kk in range(nk):
                            rhs = sc["xsrc"][kk](k, a0, a1)
                            nc.tensor.matmul(xpa[:, j], _wih_tile(sc, kk, j), rhs,
                                             start=(j == 0 and kk == 0),
                                             stop=(not sc["brow"] and j == 7
                                                   and kk == nk - 1))
                        if sc["brow"]:
                            nc.tensor.matmul(
                                xpa[:, j], sc["brow"][0][:, sc["brow"][1] + j, :],
                                ones[:, 0:sub * NL], start=False, stop=(j == 7))
                    dst = (xsb[:, :, :, a0:a1] if bt else xsb[:, :, a0:a1, :])
                    nc.vector.tensor_copy(dst, xpa)
                else:
                    for j in range(j0, j1):
                        # full bank regardless of CB: an accumulation chain's
                        # start=True lazily zeroes its whole 2KB zero-region
                        xp = psum_x.tile([128, 512], FP, tag="xp")
                        for kk in range(nk):
                            rhs = sc["xsrc"][kk](k, a0, a1)
                            nc.tensor.matmul(xp[:, 0:sub * NL],
                                             _wih_tile(sc, kk, j), rhs,
                                             start=(kk == 0),
                                             stop=False if sc["brow"] else (kk == nk - 1))
                        if sc["brow"]:
                            nc.tensor.matmul(xp[:, 0:sub * NL],
                                             sc["brow"][0][:, sc["brow"][1] + j, :],
                                             ones[:, 0:sub * NL], start=False, stop=True)
                        dst = (xsb[:, j, :, a0:a1] if bt
                               else xsb[:, j, a0:a1, :])
                        if j % 2 == 0:
                            nc.scalar.copy(dst, xp[:, 0:sub * NL])
                        else:
                            nc.vector.tensor_copy(dst, xp[:, 0:sub * NL])

            def xtilde_thunks(sc, k):
                """(engine_hint, thunk) list for chunk k's projection, each
                small enough (~1 psum->sbuf copy) to hide in a step's idle
                window on that engine.  Allocates the destination buffer
                eagerly so thunk emission order within the phase is free."""
                xsb = sc["pool"].tile(
                    [128, 8, NL, C] if sc.get("bt") else [128, 8, C, NL],
                    BF, tag=f"x_{sc['tag']}", bufs=sc.get("xbufs", 1),
                    name=f"x_{sc['tag']}")
                sc["xsb_next"] = xsb
                th = [(sc.get("h0", "d"),
                       lambda sc=sc, k=k, xsb=xsb:
                       emit_xtilde_part(sc, k, 0, xsb=xsb))]
                for r in range(1, len(XSPL) - 1):
                    for j in range(8):
                        th.append(("a" if j % 2 == 0 else "d",
                                   lambda sc=sc, k=k, r=r, j=j, xsb=xsb:
                                   emit_xtilde_part(sc, k, r, j, j + 1, xsb=xsb)))
                return th

            def xtilde_commit(sc, k):
                sc["cur_chunk"] = k
                sc["xsb"] = sc["xsb_next"]

            # =======================================================
            # ENCODER
            # =======================================================
            with ExitStack() as ectx:
                epool = ectx.enter_context(tc.tile_pool(name="enc", bufs=1))
                psg_e = ectx.enter_context(
                    tc.tile_pool(name="psge", bufs=1, space="PSUM"))
                ew_ih = epool.tile([128, 128, 128], BF)
                ew_hh = epool.tile([128, 64, 128], BF)
                ew_b = epool.tile([1, 32, 128], BF)
                nc.sync.dma_start(out=ew_ih, in_=d_ewih)
                nc.sync.dma_start(out=ew_hh, in_=d_ewhh)
                nc.sync.dma_start(out=ew_b, in_=d_eb)

                eo_bf = epool.tile([128, 4, S, NL], BF)
                for kk in range(4):
                    nc.sync.dma_start(out=eo_bf[:, kk], in_=d_encrhs[kk])

                escan = {}
                for (l, d) in [(0, 0), (0, 1), (1, 0), (1, 1)]:
                    tag = f"e{l}{d}"
                    hseq = epool.tile([128, S + 1, 2, NL], BF, tag=f"hs_{tag}")
                    init_slot = 0 if d == 0 else S
                    nc.vector.memset(hseq[:, init_slot], 0.0)
                    widx = ((l * 2 + d) * 2) * 8
                    wxidx = ((l * 2 + d) * 4) * 8
                    bidx = (l * 2 + d) * 8
                    if l == 0:
                        xsrc = []
                        for kk in range(4):
                            def f(k, a0, a1, kk=kk, d=d):
                                tr0 = k * C if d == 0 else S - (k + 1) * C
                                return eo_bf[:, kk, tr0 + a0:tr0 + a1, :]
                            xsrc.append(f)
                    else:
                        xsrc = []
                        for kk in range(4):
                            def f(k, a0, a1, kk=kk, d=d):
                                tr0 = k * C if d == 0 else S - (k + 1) * C
                                if kk < 2:
                                    return escan["e00"]["hseq"][:, tr0 + 1 + a0:tr0 + 1 + a1, kk, :]
                                else:
                                    return escan["e01"]["hseq"][:, tr0 + a0:tr0 + a1, kk - 2, :]
                            xsrc.append(f)
                    escan[tag] = dict(
                        tag=tag, fwd=(d == 0), S=S, hseq=hseq, pool=epool,
                        whh=(ew_hh, widx), wih=(ew_ih, wxidx, True),
                        brow=(ew_b, bidx), xsrc=xsrc, cur_chunk=0, xbufs=2)

                # one singleton group per scan; fwd/bwd pipeline as A/B
                egroups = {}
                for (l, d) in [(0, 0), (0, 1), (1, 0), (1, 1)]:
                    tag = f"e{l}{d}"
                    egroups[tag] = make_group(tag + "g", [escan[tag]], epool,
                                              psg_e, ptag="AB"[d])
                    nc.vector.memset(egroups[tag]["cst"], 0.0)

                # layer-l chunk-(k+1) projections depend only on eo_bf (l=0)
                # or the fully-written L0 hseqs (l=1), so they pre-emit
                # inside chunk k's steps; only each layer's first chunk pays
                # the projection latency up front.
                for l in range(2):
                    gf, gb = egroups[f"e{l}0"], egroups[f"e{l}1"]
                    scans = (gf["scans"][0], gb["scans"][0])
                    for k in range(SC):
                        urgent = []
                        if k == 0:
                            for _, th in [t for sc in scans
                                          for t in xtilde_thunks(sc, 0)]:
                                th()
                        for sc in scans:
                            xtilde_commit(sc, k)
                        pre = []
                        if k + 1 < SC:
                            pre = [t for sc in scans
                                   for t in xtilde_thunks(sc, k + 1)]
                        run_phase([(gf, gf["scans"]), (gb, gb["scans"])],
                                  pre=pre, urgent=urgent)

                # copy finals into persistent init tiles
                fin = ["e00", "e01", "e10", "e11"]
                for li, tag in enumerate(fin):
                    slot = S if tag.endswith("0") else 0
                    nc.vector.tensor_copy(hinit[li], escan[tag]["hseq"][:, slot])
                    nc.vector.tensor_copy(cinit[li], egroups[tag]["cst"][:, 0])

            # =======================================================
            # DECODER (4-layer chunk-lagged wavefront, 2 groups of 2)
            # =======================================================
            with ExitStack() as dctx:
                dpool = dctx.enter_context(tc.tile_pool(name="dec", bufs=1))
                psg_d = dctx.enter_context(
                    tc.tile_pool(name="psgd", bufs=1, space="PSUM"))
                dw_ih0 = dpool.tile([97, 8, 128], BF)
                dw_ih = dpool.tile([128, 48, 128], BF)
                dw_b = dpool.tile([1, 24, 128], BF)
                dw_hh = dpool.tile([128, 64, 128], BF)
                fw = dpool.tile([128, 2, NMEL], BF)
                fb = dpool.tile([1, NMEL], BF)
                nc.sync.dma_start(out=dw_ih0, in_=d_dwih0)
                nc.sync.dma_start(out=dw_ih, in_=d_dwih)
                nc.sync.dma_start(out=dw_b, in_=d_db)
                nc.sync.dma_start(out=dw_hh, in_=d_dwhh)
                nc.sync.dma_start(out=fw, in_=d_fcw)
                nc.sync.dma_start(out=fb, in_=d_fcb)

                teach = dpool.tile([97, NL, C], BF)
                nc.vector.memset(teach, 0.0)
                nc.vector.memset(teach[96:97], 1.0)
                mst = dpool.tile([NMEL, NL, C], FP)
                icst = dpool.tile([NMEL, 1], FP)

                # shared hseq mega-tiles: layer 2g+1's view is shifted +C
                # slots so both group members write one physical slot/step
                hq_t = [dpool.tile([128, TD + 1 + C, 2, 2, NL], BF,
                                   tag=f"hq{gi}", name=f"hq{gi}")
                        for gi in range(2)]

                dscan = []
                for l in range(4):
                    tag = f"d{l}"
                    a = l % 2
                    hseq = hq_t[l // 2][:, a * C:a * C + TD + 1, a, :, :]
                    nc.vector.tensor_copy(hseq[:, 0], hinit[l])
                    if l == 0:
                        wih = (dw_ih0, 0, False)
                        brow = None
                        xsrc = [lambda k, a0, a1: teach[:, :, a0:a1]]
                    else:
                        wih = (dw_ih, (l - 1) * 16, True)
                        brow = (dw_b, (l - 1) * 8)
                        xsrc = []
                        for kk in range(2):
                            def f(k, a0, a1, kk=kk, l=l):
                                t0 = k * C
                                return dscan[l - 1]["hseq"][:, t0 + 1 + a0:t0 + 1 + a1, kk, :]
                            xsrc.append(f)
                    dscan.append(dict(
                        tag=tag, fwd=True, S=TD, hseq=hseq, pool=dpool,
                        whh=(dw_hh, l * 16), wih=wih, brow=brow, xsrc=xsrc,
                        bt=(l == 0), cur_chunk=0, xbufs=2 if l == 0 else 1))

                dgroups = [
                    make_group("d0", [dscan[0], dscan[1]], dpool, psg_d, "A",
                               shared_hseq=hq_t[0]),
                    make_group("d1", [dscan[2], dscan[3]], dpool, psg_d, "B",
                               shared_hseq=hq_t[1]),
                ]
                for gi, g in enumerate(dgroups):
                    for a in range(2):
                        nc.vector.tensor_copy(g["cst"][:, a], cinit[gi * 2 + a])

                mels_r = d_mels.rearrange("n c t -> c n t")  # [80, NL, T]

                def fill_teacher(k):
                    t0 = k * C
                    if k == 0:
                        nc.sync.dma_start(out=icst, in_=d_ictx.rearrange("(c o) -> c o", o=1))
                        nc.vector.tensor_copy(
                            mst[:, :, 0], icst.to_broadcast((NMEL, NL)))
                        nc.sync.dma_start(
                            out=mst[:, :, 1:], in_=mels_r[:, :, 0:C - 1])
                    else:
                        nc.sync.dma_start(
                            out=mst, in_=mels_r[:, :, t0 - 1:t0 + C - 1])
                    nc.vector.tensor_copy(teach[0:NMEL, :], mst)

                def emit_fc(k):
                    t0 = k * C
                    fpt = psum_x.tile([NMEL, 512], FP, tag="xp", name=f"fcp{k}")
                    fp = fpt[:, 0:CB]
                    for kk in range(2):
                        nc.tensor.matmul(
                            fp, fw[:, kk, :],
                            dscan[3]["hseq"][:, t0 + 1:t0 + C + 1, kk, :],
                            start=(kk == 0), stop=False)
                    nc.tensor.matmul(fp, fb, ones, start=False, stop=True)
                    fst = stash.tile([NMEL, NL, C], FP, tag="fst", name=f"fst{k}")
                    nc.scalar.copy(fst, fp.rearrange("p (t b) -> p b t", b=NL))
                    if k == DC - 1:
                        nc.vector.memset(fst[:, :, C - 1:C], 0.0)
                    nc.sync.dma_start(out=out_r[:, :, t0:t0 + C], in_=fst)

                # wavefront phases; layer-0 chunk p+1 projections (teacher:
                # input-only) pre-emit inside phase p; layers 1-3 need the
                # previous layer's chunk, so theirs stay at the boundary
                for p in range(DC + 3):
                    active = [l for l in range(4) if 0 <= p - l < DC]
                    urgent = []
                    if p == 0:
                        fill_teacher(0)
                        for _, th in xtilde_thunks(dscan[0], 0):
                            th()
                    for l in active:
                        if l >= 1:
                            for _, th in xtilde_thunks(dscan[l], p - l):
                                th()
                    for l in active:
                        xtilde_commit(dscan[l], p - l)
                    pre = []
                    if p + 1 < DC:
                        pre = ([("d", lambda p=p: fill_teacher(p + 1))]
                               + xtilde_thunks(dscan[0], p + 1))
                    groups_active = []
                    for gi, g in enumerate(dgroups):
                        actset = [dscan[l] for l in active if l // 2 == gi]
                        if actset:
                            groups_active.append((g, actset))
                    run_phase(groups_active, pre=pre, urgent=urgent)
                    if 0 <= p - 4 < DC:
                        emit_fc(p - 4)

                fc_done = {p - 4 for p in range(DC + 3) if 0 <= p - 4 < DC}
                for k in range(DC):
                    if k not in fc_done:
                        emit_fc(k)

    nc.compile()
    return nc


def _host_prep(inputs):
    """Slice batch across cores + pre-tile/cast weights (gate order i,f,o,g).
    Returns in_maps."""
    bf16 = ml_dtypes.bfloat16

    def tiles_T(w, kchunks, jchunks):
        # w: [4H, D] fp32 -> [k, j, 128, 128] bf16 tiles of w.T, j in
        # GATE_PERM order (i,f,o,g); g-gate tiles (j >= 6) scaled by 2 so
        # tanh(x) = 2*sigmoid(2x)-1 turns the whole gate bank into one
        # sigmoid (scale in fp32, before the bf16 cast)
        wT = np.ascontiguousarray(w.T.astype(np.float32))  # [D, 4H]
        out = np.zeros((kchunks, jchunks, 128, 128), bf16)
        for k in range(kchunks):
            for j in range(jchunks):
                jo = GATE_PERM[j]
                t = wT[k * 128:(k + 1) * 128, jo * 128:(jo + 1) * 128]
                out[k, j] = ((t * 2.0) if j >= 6 else t).astype(bf16)
        return out

    def perm_b(b):
        # b: [4H] -> [8, 128] rows in GATE_PERM order, g rows scaled by 2
        r = b.reshape(8, 128).astype(np.float32)[GATE_PERM].copy()
        r[6:8] *= 2.0
        return r

    enc_Wih = np.asarray(inputs["enc_Wih"], np.float32)
    enc_Whh = np.asarray(inputs["enc_Whh"], np.float32)
    enc_b = np.asarray(inputs["enc_b"], np.float32)
    dec_Wih0 = np.asarray(inputs["dec_Wih0"], np.float32)
    dec_Wih = np.asarray(inputs["dec_Wih"], np.float32)
    dec_Whh = np.asarray(inputs["dec_Whh"], np.float32)
    dec_b = np.asarray(inputs["dec_b"], np.float32)
    fc_W = np.asarray(inputs["fc_W"], np.float32)
    fc_b = np.asarray(inputs["fc_b"], np.float32)
    ictx = np.asarray(inputs["init_ctx"], np.float32).reshape(-1)

    ewih = np.zeros((2, 2, 4, 8, 128, 128), bf16)
    ewhh = np.zeros((2, 2, 2, 8, 128, 128), bf16)
    eb = np.zeros((2, 2, 8, 128), bf16)
    for l in range(2):
        for d in range(2):
            ewih[l, d] = tiles_T(enc_Wih[l, d], 4, 8)
            ewhh[l, d] = tiles_T(enc_Whh[l, d], 2, 8)
            eb[l, d] = perm_b(enc_b[l, d]).astype(bf16)

    dwih0 = np.zeros((97, 8, 128), bf16)
    w0T = dec_Wih0.T.astype(np.float32)  # [80, 1024]
    db0 = perm_b(dec_b[0])
    for j in range(8):
        jo = GATE_PERM[j]
        t = w0T[:, jo * 128:(jo + 1) * 128]
        dwih0[0:80, j] = ((t * 2.0) if j >= 6 else t).astype(bf16)
        dwih0[96, j] = db0[j].astype(bf16)

    dwih = np.zeros((3, 2, 8, 128, 128), bf16)
    db = np.zeros((3, 8, 128), bf16)
    for l in range(3):
        dwih[l] = tiles_T(dec_Wih[l], 2, 8)
        db[l] = perm_b(dec_b[l + 1]).astype(bf16)
    dwhh = np.zeros((4, 2, 8, 128, 128), bf16)
    for l in range(4):
        dwhh[l] = tiles_T(dec_Whh[l], 2, 8)

    fcw = np.zeros((2, 128, NMEL), bf16)
    fWT = fc_W.T.astype(bf16)  # [256, 80]
    fcw[0] = fWT[0:128]
    fcw[1] = fWT[128:256]

    encout = np.asarray(inputs["encoder_outputs"], np.float32)
    mels = np.asarray(inputs["mels"], np.float32)
    N = encout.shape[0]
    nb = N // NCORES

    base = {
        "ictx": ictx,
        "ewih": np.ascontiguousarray(ewih.reshape(128, 128, 128).transpose(1, 0, 2)),
        "ewhh": np.ascontiguousarray(ewhh.reshape(64, 128, 128).transpose(1, 0, 2)),
        "eb": np.ascontiguousarray(eb.reshape(1, 32, 128)),
        "dwih0": dwih0,
        "dwih": np.ascontiguousarray(dwih.reshape(48, 128, 128).transpose(1, 0, 2)),
        "db": np.ascontiguousarray(db.reshape(1, 24, 128)),
        "dwhh": np.ascontiguousarray(dwhh.reshape(64, 128, 128).transpose(1, 0, 2)),
        "fcw": np.ascontiguousarray(fcw.transpose(1, 0, 2)),
        "fcb": fc_b.astype(bf16).reshape(1, NMEL),
    }
    S = encout.shape[1]
    in_maps = []
    for cid in range(NCORES):
        m = dict(base)
        eo = encout[cid * nb:(cid + 1) * nb]  # [nb, S, 512]
        m["encrhs"] = np.ascontiguousarray(
            eo.transpose(2, 1, 0).reshape(4, 128, S, nb).astype(bf16))
        m["mels"] = np.ascontiguousarray(mels[cid * nb:(cid + 1) * nb])
        in_maps.append(m)
    return in_maps


def kernel(encoder_outputs, mels, text_lengths, output_lengths,
           enc_Wih, enc_Whh, enc_b, dec_Wih0, dec_Wih, dec_Whh, dec_b,
           fc_W, fc_b, init_ctx):
    from concourse import bass_utils

    inputs = dict(encoder_outputs=encoder_outputs, mels=mels,
                  enc_Wih=enc_Wih, enc_Whh=enc_Whh, enc_b=enc_b,
                  dec_Wih0=dec_Wih0, dec_Wih=dec_Wih, dec_Whh=dec_Whh,
                  dec_b=dec_b, fc_W=fc_W, fc_b=fc_b, init_ctx=init_ctx)
    N, S, _ = np.asarray(encoder_outputs).shape
    T = np.asarray(mels).shape[2]
    key = (S, T)
    if key not in _prog_cache:
        _prog_cache[key] = _build_program(S, T)
    nc = _prog_cache[key]
    in_maps = _host_prep(inputs)
    res = bass_utils.run_bass_kernel_spmd(nc, in_maps, core_ids=list(range(NCORES)))
    nb = N // NCORES
    out = np.zeros((N, NMEL, T), np.float32)
    for cid in range(NCORES):
        out[cid * nb:(cid + 1) * nb] = res.results[cid]["out"]
    return (out,)
